# revision 1
# baseline (speedup 1.0000x reference)
"""Trainium2 Bass kernel for nn_GaussianBasis (2D gaussian-splat sum rasterizer).

Math: out[c,d,h,w] = sum_n opacity_n * exp(-sigma_n(h,w)) * features[c,n,d]
where sigma is a per-gaussian quadratic form in pixel coords.

Strategy:
  - Gaussians have tiny support (std <= ~1.8px, 6-sigma radius <= ~11px), so
    bin them host-side into 32x32-pixel buckets (8 h-bands x 8 w-cols) with a
    sigma <= SIG_CUT cutoff ellipse; contributions outside vanish in fp32.
  - sigma over a bucket is a K=6 matmul: sigma[k,px] = W6[:,k]^T @ phi[:,px],
    phi = [x^2, y^2, x*y, x, y, 1] in bucket-CENTERED coords. With |x|,|y| <=
    15.5 every phi entry is a quarter-integer <= 240.25 — exactly
    representable in fp16. W6 is split hi/lo into two fp16 halves and both
    matmuls fold into ONE K=12 fp16 matmul (1 cycle/row on PE vs 4 for fp32).
  - Each of the 8 cores owns one 32-row h-band: per col-bucket, PE computes
    sigma (K=12 fp16 matmul, fp32 PSUM) -> ACT computes g=exp(-sigma)
    PSUM->SBUF (fp16 out, 1024px per instr) -> PE computes the feature einsum
    (fp16 matmul, fp32 PSUM accumulate) -> DMA the PSUM accumulator straight
    to the output band. No collectives: pixel-sharding keeps outputs disjoint.
"""

import sys
import os

sys.path.insert(0, "/opt/trn_rl_repo")

import numpy as np
from contextlib import ExitStack

N, C, H, W = 2048, 16, 256, 256
NCORES = 8
BH, BW = 32, 32               # bucket (tile) size in pixels
NBH, NBW = H // BH, W // BW   # 8 h-bands (one per core), 8 w-cols
PX = BH * BW                  # 1024 pixels per bucket
CHUNK = 512                   # pixels per matmul (one PSUM bank of fp32 out)
NCH = PX // CHUNK             # 2 chunks per bucket
SIG_CUT = 18.0                # exp(-18) ~ 1.5e-8: negligible vs output scale

_cached = {}


def _host_prep(xyz_raw, cholesky_raw, features, opacity):
    """Bin gaussians into (band, col) buckets; emit per-bucket quadratic
    coefficients (bucket-centered coords, fp16 hi/lo split) and
    opacity-folded feature matrices."""
    xy = np.tanh(xyz_raw.astype(np.float64))
    cx = 0.5 * (xy[:, 0] + 1.0) * W
    cy = 0.5 * (xy[:, 1] + 1.0) * H
    chol = cholesky_raw.astype(np.float64) + np.array([0.5, 0.0, 0.5])
    l1, l2, l3 = chol[:, 0], chol[:, 1], chol[:, 2]
    a = l1 * l1
    b = l1 * l2
    c = l2 * l2 + l3 * l3
    det = a * c - b * b
    Aq = 0.5 * (c / det)      # coeff of dx^2
    Bq = -b / det             # coeff of dx*dy
    Cq = 0.5 * (a / det)      # coeff of dy^2
    # ellipse {sigma <= SIG_CUT} axis-aligned bounding half-widths
    rx = np.sqrt(2.0 * SIG_CUT * a) + 1.0
    ry = np.sqrt(2.0 * SIG_CUT * c) + 1.0

    featw = features.astype(np.float64) * opacity[:, 0][None, :, None]  # [C,N,3]
    featw = np.transpose(featw, (1, 0, 2)).reshape(N, C * 3)            # [N,48]

    buckets = [[[] for _ in range(NBW)] for _ in range(NBH)]
    h_lo = np.floor(cy - ry).astype(int)
    h_hi = np.ceil(cy + ry).astype(int)
    w_lo = np.floor(cx - rx).astype(int)
    w_hi = np.ceil(cx + rx).astype(int)
    for n in range(N):
        for bh in range(max(0, h_lo[n] // BH), min(NBH, h_hi[n] // BH + 1)):
            for bw in range(max(0, w_lo[n] // BW), min(NBW, w_hi[n] // BW + 1)):
                buckets[bh][bw].append(n)

    kmax = max(len(buckets[i][j]) for i in range(NBH) for j in range(NBW))
    NT = max(1, (kmax + 127) // 128)
    K_pad = NT * 128

    # Arrays laid out exactly as the SBUF tiles expect, so each input is ONE
    # contiguous DMA: w12 [12, NBW*K_pad], feat [128, NBW*NT*48].
    w12 = np.zeros((NBH, 12, NBW * K_pad), dtype=np.float16)
    feat = np.zeros((NBH, 128, NBW * NT * 48), dtype=np.float16)
    for bh in range(NBH):
        for bw in range(NBW):
            ns = np.array(buckets[bh][bw], dtype=int)
            k = len(ns)
            if k == 0:
                continue
            cxl = cx[ns] - bw * BW - BW / 2
            cyl = cy[ns] - bh * BH - BH / 2
            An, Bn, Cn = Aq[ns], Bq[ns], Cq[ns]
            W6 = np.stack(
                [
                    An,
                    Cn,
                    Bn,
                    -(2.0 * An * cxl + Bn * cyl),
                    -(2.0 * Cn * cyl + Bn * cxl),
                    An * cxl * cxl + Cn * cyl * cyl + Bn * cxl * cyl,
                ],
                0,
            )
            W_hi = W6.astype(np.float16)
            W_lo = (W6 - W_hi.astype(np.float64)).astype(np.float16)
            w12[bh, :6, bw * K_pad:bw * K_pad + k] = W_hi
            w12[bh, 6:, bw * K_pad:bw * K_pad + k] = W_lo
            fk = featw[ns].astype(np.float16)            # [k, 48]
            for nt in range((k + 127) // 128):
                p = min(128, k - nt * 128)
                feat[bh, :p, (bw * NT + nt) * 48:(bw * NT + nt + 1) * 48] = \
                    fk[nt * 128:nt * 128 + p]

    # bucket-centered pixel coords: every entry a quarter-integer <= 240.25,
    # exact in fp16
    xs = (np.arange(BW) + 0.5 - BW / 2).astype(np.float32)
    ys = (np.arange(BH) + 0.5 - BH / 2).astype(np.float32)
    Yg, Xg = np.meshgrid(ys, xs, indexing="ij")
    phi6 = np.stack(
        [Xg * Xg, Yg * Yg, Xg * Yg, Xg, Yg, np.ones_like(Xg)], 0
    ).reshape(6, PX)
    phi12 = np.concatenate([phi6, phi6], 0).astype(np.float16)  # [12, PX]
    return w12, feat, phi12, NT


def _build_program(NT):
    import concourse.bacc as bacc
    import concourse.tile as tile
    import concourse.mybir as mybir

    nc = bacc.Bacc("TRN2", target_bir_lowering=False, debug=False,
                   num_devices=NCORES)
    KP = NT * 128
    w12_ap = nc.dram_tensor("w12", [12, NBW * KP], mybir.dt.float16,
                            kind="ExternalInput").ap()
    feat_ap = nc.dram_tensor("feat", [128, NBW * NT * 48], mybir.dt.float16,
                             kind="ExternalInput").ap()
    phi_ap = nc.dram_tensor("phi", [12, PX], mybir.dt.float16,
                            kind="ExternalInput").ap()
    out_ap = nc.dram_tensor("out", [C * 3, BH, W], mybir.dt.float32,
                            kind="ExternalOutput").ap()

    HB = BH // NCH  # h-rows per chunk (16)
    with tile.TileContext(nc) as tc:
        with ExitStack() as ctx:
            consts = ctx.enter_context(tc.tile_pool(name="consts", bufs=1))
            spool = ctx.enter_context(
                tc.tile_pool(name="sig", bufs=3, space="PSUM"))
            opool = ctx.enter_context(
                tc.tile_pool(name="acc", bufs=2, space="PSUM"))
            gpool = ctx.enter_context(tc.tile_pool(name="g", bufs=3))

            # PE HAM warmup: dummy matmuls on a zeroed SBUF tile while the
            # input DMAs are in flight, so real matmuls start at 2.4 GHz.
            # They rotate through the same psum_s slots as the real sigma
            # matmuls (same tag), serializing only on PE, which is idle.
            dummy = consts.tile([12, 640], mybir.dt.float16)
            nc.vector.memset(dummy, 0)
            for _ in range(2):
                psum_s = spool.tile([128, PX], mybir.dt.float32)
                nc.tensor.matmul(psum_s[:, 0:CHUNK], dummy[:, 0:128],
                                 dummy[:, 128:640], start=True, stop=True)

            # inputs: one contiguous DMA each; phi+w12 on the SP HWDGE queue
            # (ACT's queue is busy with the exp table load), feat on SWDGE
            phi_sb = consts.tile([12, PX], mybir.dt.float16)
            nc.sync.dma_start(out=phi_sb, in_=phi_ap)
            w12_sb = consts.tile([12, NBW * KP], mybir.dt.float16)
            nc.sync.dma_start(out=w12_sb, in_=w12_ap)
            feat_sb = consts.tile([128, NBW * NT * 48], mybir.dt.float16)
            nc.gpsimd.dma_start(out=feat_sb, in_=feat_ap)

            # final band accumulator in SBUF: partitions [0:48] hold chunk 0
            # (h 0..15), [64:112] chunk 1 (h 16..31); free dim is the DRAM
            # band layout (h-major, w global) so the output DMA is contiguous
            out_sb = consts.tile([112, HB * W], mybir.dt.float32)

            for col in range(NBW):
                psum_o = opool.tile([112, CHUNK], mybir.dt.float32)
                for nt in range(NT):
                    psum_s = spool.tile([128, PX], mybir.dt.float32)
                    for ch in range(NCH):
                        nc.tensor.matmul(
                            psum_s[:, ch * CHUNK:(ch + 1) * CHUNK],
                            w12_sb[:, (col * NT + nt) * 128:(col * NT + nt + 1) * 128],
                            phi_sb[:, ch * CHUNK:(ch + 1) * CHUNK],
                            start=True, stop=True)
                    g = gpool.tile([128, PX], mybir.dt.float16)
                    nc.scalar.activation(
                        g, psum_s, mybir.ActivationFunctionType.Exp,
                        bias=0.0, scale=-1.0)
                    for ch in range(NCH):
                        nc.tensor.matmul(
                            psum_o[64 * ch:64 * ch + 48, :],
                            feat_sb[:, (col * NT + nt) * 48:(col * NT + nt + 1) * 48],
                            g[:, ch * CHUNK:(ch + 1) * CHUNK],
                            start=(nt == 0), stop=(nt == NT - 1),
                            tile_position=(0, 64 * ch))
                nc.vector.tensor_copy(
                    out_sb.rearrange("p (h cw) -> p h cw", cw=W)[
                        :, :, col * BW:(col + 1) * BW],
                    psum_o.rearrange("p (h w) -> p h w", w=BW))

            # two contiguous output DMAs: partitions [0:48] -> h rows 0..15,
            # [64:112] -> h rows 16..31
            for ch in range(NCH):
                nc.sync.dma_start(
                    out=out_ap[:, ch * HB:(ch + 1) * HB, :],
                    in_=out_sb[64 * ch:64 * ch + 48, :].rearrange(
                        "p (h cw) -> p h cw", cw=W))
    nc.compile()
    return nc


def _host_prep_packed(cx, cy, Aq, Bq, Cq, rx, ry, featw):
    """16x16-px buckets, two vertical halves packed per 128-partition tile
    (top half-band -> partitions 0:64, bottom -> 64:128). Requires every
    bucket to hold <= 64 gaussians; returns None if not."""
    BH2 = BW2 = 16
    ncol = W // BW2                       # 16 cols per band
    nrow = H // BH2                       # 16 half-band rows
    buckets = [[[] for _ in range(ncol)] for _ in range(nrow)]
    h_lo = np.floor(cy - ry).astype(int)
    h_hi = np.ceil(cy + ry).astype(int)
    w_lo = np.floor(cx - rx).astype(int)
    w_hi = np.ceil(cx + rx).astype(int)
    for n in range(N):
        for bh in range(max(0, h_lo[n] // BH2), min(nrow, h_hi[n] // BH2 + 1)):
            for bw in range(max(0, w_lo[n] // BW2), min(ncol, w_hi[n] // BW2 + 1)):
                buckets[bh][bw].append(n)
    if max(len(buckets[i][j]) for i in range(nrow) for j in range(ncol)) > 64:
        return None

    PX2 = BH2 * BW2
    w12 = np.zeros((NCORES, 12, PX2 + ncol * 128), dtype=np.float16)
    feat = np.zeros((NCORES, 128, ncol * 48), dtype=np.float16)
    for core in range(NCORES):
        for col in range(ncol):
            for half in range(2):
                ns = np.array(buckets[2 * core + half][col], dtype=int)
                k = len(ns)
                if k == 0:
                    continue
                cxl = cx[ns] - col * BW2 - BW2 / 2
                cyl = cy[ns] - (2 * core + half) * BH2 - BH2 / 2
                An, Bn, Cn = Aq[ns], Bq[ns], Cq[ns]
                W6 = np.stack(
                    [
                        An,
                        Cn,
                        Bn,
                        -(2.0 * An * cxl + Bn * cyl),
                        -(2.0 * Cn * cyl + Bn * cxl),
                        An * cxl * cxl + Cn * cyl * cyl + Bn * cxl * cyl,
                    ],
                    0,
                )
                W_hi = W6.astype(np.float16)
                W_lo = (W6 - W_hi.astype(np.float64)).astype(np.float16)
                base = PX2 + col * 128 + 64 * half
                w12[core, :6, base:base + k] = W_hi
                w12[core, 6:, base:base + k] = W_lo
                feat[core, 64 * half:64 * half + k, col * 48:col * 48 + 48] = \
                    featw[ns].astype(np.float16)

    xs = (np.arange(BW2) + 0.5 - BW2 / 2).astype(np.float32)
    ys = (np.arange(BH2) + 0.5 - BH2 / 2).astype(np.float32)
    Yg, Xg = np.meshgrid(ys, xs, indexing="ij")
    phi6 = np.stack(
        [Xg * Xg, Yg * Yg, Xg * Yg, Xg, Yg, np.ones_like(Xg)], 0
    ).reshape(6, BH2 * BW2)
    phi12 = np.concatenate([phi6, phi6], 0).astype(np.float16)  # [12, 256]
    w12[:, :, 0:PX2] = phi12[None]
    return w12, feat, phi12


def _build_program_packed():
    import concourse.bacc as bacc
    import concourse.tile as tile
    import concourse.mybir as mybir

    BH2 = BW2 = 16
    ncol = W // BW2                 # 16 packed tiles per core
    PX2 = BH2 * BW2                 # 256 px per bucket
    npair = ncol // 2               # col pairs sharing one PSUM/ACT group

    nc = bacc.Bacc("TRN2", target_bir_lowering=False, debug=False,
                   num_devices=NCORES)
    # phi rides in the same tensor as w12 (FIRST PX2 columns), so the first
    # DMA chunk (phi + first 4 col tiles) lands before the rest
    w12_ap = nc.dram_tensor("w12", [12, PX2 + ncol * 128], mybir.dt.float16,
                            kind="ExternalInput").ap()
    feat_ap = nc.dram_tensor("feat", [128, ncol * 48], mybir.dt.float16,
                             kind="ExternalInput").ap()
    out_ap = nc.dram_tensor("out", [C * 3, BH, W], mybir.dt.float32,
                            kind="ExternalOutput").ap()

    with tile.TileContext(nc) as tc:
        with ExitStack() as ctx:
            consts = ctx.enter_context(tc.tile_pool(name="consts", bufs=1))
            spool = ctx.enter_context(
                tc.tile_pool(name="sig", bufs=2, space="PSUM"))
            opool = ctx.enter_context(
                tc.tile_pool(name="acc", bufs=3, space="PSUM"))
            gpool = ctx.enter_context(tc.tile_pool(name="g", bufs=3))

            dummy = consts.tile([12, 640], mybir.dt.float16)
            nc.vector.memset(dummy, 0)
            for _ in range(2):
                psum_s = spool.tile([128, 4 * PX2], mybir.dt.float32)
                nc.tensor.matmul(psum_s[:, 0:512], dummy[:, 0:128],
                                 dummy[:, 128:640], start=True, stop=True)

            w12_sb = consts.tile([12, PX2 + ncol * 128], mybir.dt.float16)
            CUT = PX2 + 4 * 128
            nc.sync.dma_start(out=w12_sb[:, :CUT], in_=w12_ap[:, :CUT])
            nc.sync.dma_start(out=w12_sb[:, CUT:], in_=w12_ap[:, CUT:])
            phi_sb = w12_sb[:, 0:PX2]
            feat_sb = consts.tile([128, ncol * 48], mybir.dt.float16)
            nc.gpsimd.dma_start(out=feat_sb, in_=feat_ap)

            # band accumulator, h-major DRAM layout; partitions [0:48] hold
            # h 0..15, [64:112] h 16..31
            out_sb = consts.tile([112, (BH // 2) * W], mybir.dt.float32)
            out_v = out_sb.rearrange("p (h cw) -> p h cw", cw=W)

            for qr in range(npair // 2):
                # one 4-col sigma/exp group (fewer ACT instruction overheads)
                psum_s = spool.tile([128, 4 * PX2], mybir.dt.float32)
                for j in range(4):
                    t = 4 * qr + j
                    nc.tensor.matmul(
                        psum_s[:, j * PX2:(j + 1) * PX2],
                        w12_sb[:, PX2 + t * 128:PX2 + (t + 1) * 128],
                        phi_sb,
                        start=True, stop=True)
                g = gpool.tile([128, 4 * PX2], mybir.dt.float16)
                nc.scalar.activation(
                    g, psum_s, mybir.ActivationFunctionType.Exp,
                    bias=0.0, scale=-1.0)
                for pq in range(2):
                    pr = 2 * qr + pq
                    psum_o = opool.tile([112, 512], mybir.dt.float32)
                    for j in range(2):
                        t = 2 * pr + j
                        gj = 2 * pq + j
                        for half in range(2):
                            nc.tensor.matmul(
                                psum_o[64 * half:64 * half + 48,
                                       j * PX2:(j + 1) * PX2],
                                feat_sb[64 * half:64 * half + 64,
                                        t * 48:(t + 1) * 48],
                                g[64 * half:64 * half + 64,
                                  gj * PX2:(gj + 1) * PX2],
                                start=True, stop=True,
                                tile_position=(64 * half, 64 * half))
                    # psum free order (c2, h16, w16) -> out (h-major, global w)
                    nc.vector.tensor_copy(
                        out_v[:, :, pr * 2 * BW2:(pr + 1) * 2 * BW2].rearrange(
                            "p h (c w) -> p c h w", w=BW2),
                        psum_o.rearrange("p (c h w) -> p c h w",
                                         h=BH2, w=BW2))

            for ch in range(2):
                nc.sync.dma_start(
                    out=out_ap[:, ch * (BH // 2):(ch + 1) * (BH // 2), :],
                    in_=out_sb[64 * ch:64 * ch + 48, :].rearrange(
                        "p (h cw) -> p h cw", cw=W))
    nc.compile()
    return nc


def _params(np_inputs):
    """Per-gaussian params (fp64 host): centers, quadratic coeffs, cutoff
    radii, opacity-folded features."""
    xyz_raw = np.asarray(np_inputs["xyz_raw"], dtype=np.float32)
    cholesky_raw = np.asarray(np_inputs["cholesky_raw"], dtype=np.float32)
    features = np.asarray(np_inputs["features"], dtype=np.float32)
    opacity = np.asarray(np_inputs["opacity"], dtype=np.float32)
    xy = np.tanh(xyz_raw.astype(np.float64))
    cx = 0.5 * (xy[:, 0] + 1.0) * W
    cy = 0.5 * (xy[:, 1] + 1.0) * H
    chol = cholesky_raw.astype(np.float64) + np.array([0.5, 0.0, 0.5])
    l1, l2, l3 = chol[:, 0], chol[:, 1], chol[:, 2]
    a = l1 * l1
    b = l1 * l2
    c = l2 * l2 + l3 * l3
    det = a * c - b * b
    Aq, Bq, Cq = 0.5 * (c / det), -b / det, 0.5 * (a / det)
    rx = np.sqrt(2.0 * SIG_CUT * a) + 1.0
    ry = np.sqrt(2.0 * SIG_CUT * c) + 1.0
    featw = features.astype(np.float64) * opacity[:, 0][None, :, None]
    featw = np.transpose(featw, (1, 0, 2)).reshape(N, C * 3)
    return cx, cy, Aq, Bq, Cq, rx, ry, featw


def kernel(xyz_raw, cholesky_raw, features, opacity):
    from concourse.bass_utils import run_bass_kernel_spmd

    xyz_raw = np.asarray(xyz_raw, dtype=np.float32)
    cholesky_raw = np.asarray(cholesky_raw, dtype=np.float32)
    features = np.asarray(features, dtype=np.float32)
    opacity = np.asarray(opacity, dtype=np.float32)

    cx, cy, Aq, Bq, Cq, rx, ry, featw = _params({
        "xyz_raw": xyz_raw, "cholesky_raw": cholesky_raw,
        "features": features, "opacity": opacity})

    packed = _host_prep_packed(cx, cy, Aq, Bq, Cq, rx, ry, featw)
    if packed is not None:
        w12, feat, _ = packed
        if "packed" not in _cached:
            _cached["packed"] = _build_program_packed()
        nc = _cached["packed"]
        in_maps = [
            {"w12": w12[band], "feat": feat[band]} for band in range(NCORES)
        ]
    else:
        w12, feat, phi12, NT = _host_prep(
            xyz_raw, cholesky_raw, features, opacity)
        if NT not in _cached:
            _cached[NT] = _build_program(NT)
        nc = _cached[NT]
        in_maps = [
            {"w12": w12[band], "feat": feat[band], "phi": phi12}
            for band in range(NCORES)
        ]
    res = run_bass_kernel_spmd(nc, in_maps, core_ids=list(range(NCORES)))

    out = np.empty((C * 3, H, W), dtype=np.float32)
    for band in range(NCORES):
        out[:, band * BH:(band + 1) * BH, :] = res.results[band]["out"]
    return out.reshape(C, 3, H, W)



# revision 6
# speedup vs baseline: 1.0658x; 1.0658x over previous
"""Trainium2 Bass kernel for nn_GaussianBasis (2D gaussian-splat sum rasterizer).

Math: out[c,d,h,w] = sum_n opacity_n * exp(-sigma_n(h,w)) * features[c,n,d]
where sigma is a per-gaussian quadratic form in pixel coords.

Strategy (v2):
  - Gaussians have tiny support; bin them host-side into 16x16-pixel buckets
    with a sigma <= SIG_CUT cutoff ellipse (exp(-8) ~ 3.4e-4; dropped tail
    contributions stay far below the 2e-2 relative tolerance vs absmax ~2.7).
  - Each core owns a 32-row band = 2x16 buckets. Buckets are paired, then the
    pairs are bin-packed into 128-partition "tiles" (each gaussian = one
    partition row; a tile holds 2-3 pairs, sum k <= 128). sigma over a tile
    is ONE K=12 fp16 matmul against the shared bucket-centered
    phi = [x^2,y^2,xy,x,y,1] basis (hi/lo fp16 coefficient split for
    fp32-grade accuracy) -> [128 gauss, 256 px] PSUM.
  - ACT computes g = exp(-sigma) PSUM->SBUF fp16 in 2-tile groups. Packing
    cuts tiles from 16 to ~6, cutting the serial exp chain ~2.5x.
  - Feature einsum: per pair ("unit") one fp16 matmul with a block-structured
    [128, 96] weight (bucket A rows -> cols 0:48, B rows -> 48:96) -> PSUM
    [96, 256]. Exactly 16 units/core.
  - PSUM is only readable by ACT/DVE (hw rule: no DMA, no Pool). Output
    copies (PSUM fp32 -> SBUF fp16) are split: DVE inline for early tiles,
    ACT after its final exp for the last two tiles - balancing the two
    engines without stretching the exp chain. One DMA per tile to a
    strip-contiguous DRAM layout (runs >= 512B: full DMA bandwidth),
    overlapped with compute; the final tile copies/DMAs per-unit so the
    tail transfer is small. Host reassembles strips into [C,3,H,W].
  - No collectives: pixel-sharding keeps outputs disjoint.
"""

import sys

sys.path.insert(0, "/opt/trn_rl_repo")

import numpy as np
from contextlib import ExitStack

N, C, H, W = 2048, 16, 256, 256
NCORES = 8
BK = 16                      # bucket edge in pixels
PXB = BK * BK                # 256 px per bucket
NBR, NBC = (H // NCORES) // BK, W // BK   # 2 bucket-rows, 16 cols per core
SIG_CUT = 8.0                # exp(-8) ~ 3.4e-4: negligible vs tolerance

_cached = {}


def _params(xyz_raw, cholesky_raw, features, opacity):
    """Per-gaussian params (fp64 host): centers, quadratic coeffs, cutoff
    radii, opacity-folded features."""
    xy = np.tanh(xyz_raw.astype(np.float64))
    cx = 0.5 * (xy[:, 0] + 1.0) * W
    cy = 0.5 * (xy[:, 1] + 1.0) * H
    chol = cholesky_raw.astype(np.float64) + np.array([0.5, 0.0, 0.5])
    l1, l2, l3 = chol[:, 0], chol[:, 1], chol[:, 2]
    a = l1 * l1
    b = l1 * l2
    c = l2 * l2 + l3 * l3
    det = a * c - b * b
    Aq, Bq, Cq = 0.5 * (c / det), -b / det, 0.5 * (a / det)
    rx = np.sqrt(2.0 * SIG_CUT * a) + 1.0
    ry = np.sqrt(2.0 * SIG_CUT * c) + 1.0
    featw = features.astype(np.float64) * opacity[:, 0][None, :, None]
    featw = np.transpose(featw, (1, 0, 2)).reshape(N, C * 3)
    return cx, cy, Aq, Bq, Cq, rx, ry, featw


def _pack(cx, cy, rx, ry):
    """Bin gaussians into per-core 16x16 buckets, pair the buckets
    (largest with smallest, so pair sizes are balanced), then first-fit
    pack pairs into 128-partition tiles. Oversized buckets (>128) split.
    Returns tiles[core] = list of tiles; tile = list of units;
    unit = [(r, c, idx, part_offset), ...] with 1-2 blocks."""
    h_lo = np.floor(cy - ry).astype(int)
    h_hi = np.ceil(cy + ry).astype(int)
    w_lo = np.floor(cx - rx).astype(int)
    w_hi = np.ceil(cx + rx).astype(int)
    nrow, ncol = H // BK, W // BK
    buckets = [[[] for _ in range(ncol)] for _ in range(nrow)]
    for n in range(N):
        for bh in range(max(0, h_lo[n] // BK), min(nrow, h_hi[n] // BK + 1)):
            for bw in range(max(0, w_lo[n] // BK), min(ncol, w_hi[n] // BK + 1)):
                buckets[bh][bw].append(n)

    tiles_per_core = []
    for core in range(NCORES):
        blist = []
        for r in range(NBR):
            for c in range(NBC):
                ns = np.asarray(buckets[core * NBR + r][c], dtype=int)
                for s in range(0, len(ns), 128):
                    blist.append((r, c, ns[s:s + 128]))
        blist.sort(key=lambda b: -len(b[2]))
        # pair largest with smallest
        pairs = []
        lo, hi = 0, len(blist) - 1
        while lo < hi:
            pairs.append([blist[lo], blist[hi]])
            lo += 1
            hi -= 1
        if lo == hi:
            pairs.append([blist[lo]])
        pairs.sort(key=lambda p: -sum(len(b[2]) for b in p))
        # first-fit-decreasing pairs into tiles of <= 128 partitions
        tiles = []   # [used, [units]]
        for p in pairs:
            k = sum(len(b[2]) for b in p)
            for t in tiles:
                if t[0] + k <= 128:
                    unit = [(r, c, idx, t[0] + sum(len(p[j][2]) for j in range(bi)))
                            for bi, (r, c, idx) in enumerate(p)]
                    t[1].append(unit)
                    t[0] += k
                    break
            else:
                unit = [(r, c, idx, sum(len(p[j][2]) for j in range(bi)))
                        for bi, (r, c, idx) in enumerate(p)]
                tiles.append([k, [unit]])
        tiles.sort(key=lambda t: -len(t[1]))   # most units first (U aligns)
        tiles_per_core.append([t[1] for t in tiles])
    return tiles_per_core


def _host_prep(np_inputs):
    """Build per-core device arrays:
      w12  [12, 256 + NT*128] fp16  (cols 0:256 = phi basis, then per-tile
                                     hi/lo quadratic coefficient columns)
      feat [128, NU*96]       fp16  (per-unit block feature weights)
    plus the placement map for host-side reassembly."""
    cx, cy, Aq, Bq, Cq, rx, ry, featw = _params(
        np_inputs["xyz_raw"], np_inputs["cholesky_raw"],
        np_inputs["features"], np_inputs["opacity"])
    tiles_per_core = _pack(cx, cy, rx, ry)

    NT = max(len(t) for t in tiles_per_core)
    U = []
    for t in range(NT):
        u = 1
        for core in range(NCORES):
            if t < len(tiles_per_core[core]):
                u = max(u, len(tiles_per_core[core][t]))
        U.append(u)
    NU = sum(U)

    # bucket-centered pixel coords: quarter-integers <= 56.25, exact in fp16
    xs = (np.arange(BK) + 0.5 - BK / 2).astype(np.float64)
    Yg, Xg = np.meshgrid(xs, xs, indexing="ij")
    phi6 = np.stack(
        [Xg * Xg, Yg * Yg, Xg * Yg, Xg, Yg, np.ones_like(Xg)], 0
    ).reshape(6, PXB)
    phi12 = np.concatenate([phi6, phi6], 0).astype(np.float16)

    w12 = np.zeros((NCORES, 12, 256 + NT * 128), dtype=np.float16)
    feat = np.zeros((NCORES, 128, NU * 96), dtype=np.float16)
    w12[:, :, 0:256] = phi12[None]
    placements = []   # per core: list of (tile, unit, block, r, c)
    for core in range(NCORES):
        place = []
        uoff = 0
        for t in range(NT):
            tl = tiles_per_core[core][t] if t < len(tiles_per_core[core]) else []
            for ui, unit in enumerate(tl):
                for block, (r, c, idx, off) in enumerate(unit):
                    k = len(idx)
                    cxl = cx[idx] - c * BK - BK / 2
                    cyl = cy[idx] - (core * NBR + r) * BK - BK / 2
                    An, Bn, Cn = Aq[idx], Bq[idx], Cq[idx]
                    W6 = np.stack(
                        [
                            An,
                            Cn,
                            Bn,
                            -(2.0 * An * cxl + Bn * cyl),
                            -(2.0 * Cn * cyl + Bn * cxl),
                            An * cxl * cxl + Cn * cyl * cyl + Bn * cxl * cyl,
                        ],
                        0,
                    )
                    W_hi = W6.astype(np.float16)
                    W_lo = (W6 - W_hi.astype(np.float64)).astype(np.float16)
                    base = 256 + t * 128 + off
                    w12[core, :6, base:base + k] = W_hi
                    w12[core, 6:, base:base + k] = W_lo
                    feat[core, off:off + k,
                         (uoff + ui) * 96 + block * 48:
                         (uoff + ui) * 96 + block * 48 + 48] = \
                        featw[idx].astype(np.float16)
                    place.append((t, ui, block, r, c))
            uoff += U[t]
        placements.append(place)
    return w12, feat, NT, tuple(U), placements


def _build_program(NT, U):
    import concourse.bacc as bacc
    import concourse.tile as tile
    import concourse.mybir as mybir

    NU = sum(U)
    toff = [0]
    for u in U:
        toff.append(toff[-1] + u * 256)

    nc = bacc.Bacc("TRN2", target_bir_lowering=False, debug=False,
                   num_devices=NCORES)
    w12_ap = nc.dram_tensor("w12", [12, 256 + NT * 128], mybir.dt.float16,
                            kind="ExternalInput").ap()
    feat_ap = nc.dram_tensor("feat", [128, NU * 96], mybir.dt.float16,
                             kind="ExternalInput").ap()
    out_ap = nc.dram_tensor("out", [96, toff[-1]], mybir.dt.float16,
                            kind="ExternalOutput").ap()

    groups = [(g, min(g + 2, NT)) for g in range(0, NT, 2)]
    nuA = sum(U[t] for t in range(groups[0][0], groups[0][1]))
    NDEFER = min(2, NT)   # last tiles: copies on ACT after its final exp

    with tile.TileContext(nc) as tc:
        with ExitStack() as ctx:
            consts = ctx.enter_context(tc.tile_pool(name="consts", bufs=1))
            spool = ctx.enter_context(
                tc.tile_pool(name="sig", bufs=2, space="PSUM"))
            opool = ctx.enter_context(
                tc.tile_pool(name="acc", bufs=2, space="PSUM"))
            gpool = ctx.enter_context(tc.tile_pool(name="g", bufs=3))
            stpool = ctx.enter_context(tc.tile_pool(name="stage", bufs=3))

            # PE HAM warmup: dummy matmuls on a zeroed SBUF tile while the
            # input DMAs are in flight, so the p-state ramp completes early.
            dummy = consts.tile([12, 640], mybir.dt.float16)
            nc.vector.memset(dummy, 0)
            for _ in range(2):
                psum_d = spool.tile([128, 512], mybir.dt.float32)
                nc.tensor.matmul(psum_d, dummy[:, 0:128], dummy[:, 128:640],
                                 start=True, stop=True)

            # inputs: w12 split so phi + first-group tiles land first (SP
            # HWDGE); feat chunk A on the ACT HWDGE queue (no waits, so it
            # cannot stall exp dispatch), rest on SP after the w12 chunks.
            w12_sb = consts.tile([12, 256 + NT * 128], mybir.dt.float16)
            CUT = 256 + groups[0][1] * 128
            nc.sync.dma_start(out=w12_sb[:, :CUT], in_=w12_ap[:, :CUT])
            nc.sync.dma_start(out=w12_sb[:, CUT:], in_=w12_ap[:, CUT:])
            phi_sb = w12_sb[:, 0:256]
            feat_sb = consts.tile([128, NU * 96], mybir.dt.float16)
            nc.scalar.dma_start(out=feat_sb[:, :nuA * 96],
                                in_=feat_ap[:, :nuA * 96])
            nc.sync.dma_start(out=feat_sb[:, nuA * 96:],
                              in_=feat_ap[:, nuA * 96:])

            deferred = []
            for t0, t1 in groups:
                ntl = t1 - t0
                psum_s = spool.tile([128, ntl * 256], mybir.dt.float32)
                for j in range(ntl):
                    nc.tensor.matmul(
                        psum_s[:, j * 256:(j + 1) * 256],
                        w12_sb[:, 256 + (t0 + j) * 128:256 + (t0 + j + 1) * 128],
                        phi_sb, start=True, stop=True)
                g = gpool.tile([128, ntl * 256], mybir.dt.float16)
                nc.scalar.activation(
                    g, psum_s, mybir.ActivationFunctionType.Exp,
                    bias=0.0, scale=-1.0)
                for j in range(ntl):
                    t = t0 + j
                    psum_o = opool.tile([96, U[t] * 256], mybir.dt.float32)
                    stage = stpool.tile([96, U[t] * 256], mybir.dt.float16)
                    for u in range(U[t]):
                        nc.tensor.matmul(
                            psum_o[:, u * 256:(u + 1) * 256],
                            feat_sb[:, (toff[t] // 256 + u) * 96:
                                    (toff[t] // 256 + u + 1) * 96],
                            g[:, j * 256:(j + 1) * 256],
                            start=True, stop=True)
                    if t < NT - NDEFER:
                        nc.vector.tensor_copy(stage, psum_o)
                        nc.sync.dma_start(
                            out=out_ap[:, toff[t]:toff[t + 1]], in_=stage)
                    else:
                        deferred.append((t, psum_o, stage))

            # last tiles: ACT copies (it is done with exps; DVE still has a
            # backlog). Final tile per-unit so the tail transfer is small.
            for t, psum_o, stage in deferred:
                if t < NT - 1:
                    nc.scalar.activation(
                        stage, psum_o, mybir.ActivationFunctionType.Copy)
                    nc.sync.dma_start(
                        out=out_ap[:, toff[t]:toff[t + 1]], in_=stage)
                else:
                    for u in range(U[t]):
                        nc.scalar.activation(
                            stage[:, u * 256:(u + 1) * 256],
                            psum_o[:, u * 256:(u + 1) * 256],
                            mybir.ActivationFunctionType.Copy)
                        nc.sync.dma_start(
                            out=out_ap[:, toff[t] + u * 256:
                                       toff[t] + (u + 1) * 256],
                            in_=stage[:, u * 256:(u + 1) * 256])
    nc.compile()
    return nc


def kernel(xyz_raw, cholesky_raw, features, opacity):
    from concourse.bass_utils import run_bass_kernel_spmd

    np_inputs = {
        "xyz_raw": np.asarray(xyz_raw, dtype=np.float32),
        "cholesky_raw": np.asarray(cholesky_raw, dtype=np.float32),
        "features": np.asarray(features, dtype=np.float32),
        "opacity": np.asarray(opacity, dtype=np.float32),
    }
    w12, feat, NT, U, placements = _host_prep(np_inputs)

    key = (NT, U)
    if key not in _cached:
        _cached[key] = _build_program(NT, U)
    nc = _cached[key]

    in_maps = [{"w12": w12[core], "feat": feat[core]}
               for core in range(NCORES)]
    res = run_bass_kernel_spmd(nc, in_maps, core_ids=list(range(NCORES)))

    toff = [0]
    for u in U:
        toff.append(toff[-1] + u * 256)
    out = np.zeros((C * 3, H, W), dtype=np.float32)
    for core in range(NCORES):
        strips = res.results[core]["out"].astype(np.float32)  # [96, TOT]
        for t, unit, block, r, c in placements[core]:
            sl = strips[block * 48:block * 48 + 48,
                        toff[t] + unit * 256:toff[t] + unit * 256 + 256]
            out[:, (core * NBR + r) * BK:(core * NBR + r + 1) * BK,
                c * BK:(c + 1) * BK] += sl.reshape(48, BK, BK)
    return out.reshape(C, 3, H, W)


# revision 7
# speedup vs baseline: 1.3763x; 1.2914x over previous
"""Trainium2 Bass kernel for nn_GaussianBasis (2D gaussian-splat sum rasterizer).

Math: out[c,d,h,w] = sum_n opacity_n * exp(-sigma_n(h,w)) * features[c,n,d]
where sigma is a per-gaussian quadratic form in pixel coords.

Strategy (v3):
  - Gaussians have tiny support; bin them host-side into 16x16-pixel buckets
    with a sigma <= SIG_CUT cutoff ellipse (exp(-8) ~ 3.4e-4; dropped tail
    contributions stay far below the 2e-2 relative tolerance vs absmax ~2.7).
  - Each core owns a 32-row band = 2x16 buckets. Buckets are bin-packed
    (first-fit-decreasing) into 128-partition "tiles" (one gaussian = one
    partition row, sum k <= 128, typically 4-7 buckets). sigma over a tile is
    ONE K=12 fp16 matmul against the shared bucket-centered
    phi = [x^2,y^2,xy,x,y,1] basis (hi/lo fp16 coefficient split for
    fp32-grade accuracy) -> [128 gauss, 256 px] PSUM. Packing cuts tiles from
    16 to ~6-7, cutting the serial exp chain ~2.5x.
  - ACT computes g = exp(-sigma) PSUM->SBUF fp16 in 2-tile groups.
  - Feature einsum: per tile, the buckets' 48 output channels are laid out as
    one dense column stream chopped into 128-column "units" (channels of one
    bucket may split across units): one fp16 matmul per unit with a [128,128]
    block weight -> PSUM [128, 256] with EVERY partition carrying payload.
    This is the information-theoretic floor for PSUM evacuation volume.
  - PSUM is only readable by ACT/DVE (no DMA, no Pool). Per-tile fp32->fp16
    copies into SBUF staging are assigned greedily to whichever of ACT/DVE
    has less accumulated work (ACT starts charged with its exp chain); the
    final tile is forced to ACT (free right after its last exp) and copied
    per-unit so the tail transfer is small. One DMA per tile to a
    strip-contiguous DRAM layout (runs >= 512B: full bandwidth), overlapped
    with compute. Host reassembles strips into [C,3,H,W].
  - No collectives: pixel-sharding keeps outputs disjoint.
"""

import sys

sys.path.insert(0, "/opt/trn_rl_repo")

import numpy as np
from contextlib import ExitStack

N, C, H, W = 2048, 16, 256, 256
NCORES = 8
BK = 16                      # bucket edge in pixels
PXB = BK * BK                # 256 px per bucket
NBR, NBC = (H // NCORES) // BK, W // BK   # 2 bucket-rows, 16 cols per core
CH = C * 3                   # 48 output channels
SIG_CUT = 8.0                # exp(-8) ~ 3.4e-4: negligible vs tolerance

_cached = {}


def _params(xyz_raw, cholesky_raw, features, opacity):
    """Per-gaussian params (fp64 host): centers, quadratic coeffs, cutoff
    radii, opacity-folded features."""
    xy = np.tanh(xyz_raw.astype(np.float64))
    cx = 0.5 * (xy[:, 0] + 1.0) * W
    cy = 0.5 * (xy[:, 1] + 1.0) * H
    chol = cholesky_raw.astype(np.float64) + np.array([0.5, 0.0, 0.5])
    l1, l2, l3 = chol[:, 0], chol[:, 1], chol[:, 2]
    a = l1 * l1
    b = l1 * l2
    c = l2 * l2 + l3 * l3
    det = a * c - b * b
    Aq, Bq, Cq = 0.5 * (c / det), -b / det, 0.5 * (a / det)
    rx = np.sqrt(2.0 * SIG_CUT * a) + 1.0
    ry = np.sqrt(2.0 * SIG_CUT * c) + 1.0
    featw = features.astype(np.float64) * opacity[:, 0][None, :, None]
    featw = np.transpose(featw, (1, 0, 2)).reshape(N, CH)
    return cx, cy, Aq, Bq, Cq, rx, ry, featw


def _pack(cx, cy, rx, ry):
    """Bin gaussians into per-core 16x16 buckets; first-fit-decreasing pack
    whole buckets into 128-partition tiles (oversized buckets split).
    Returns tiles[core] = list of tiles; tile = [(r, c, idx, offset), ...]
    sorted so tiles with more channel-units come first (aligns padding)."""
    h_lo = np.floor(cy - ry).astype(int)
    h_hi = np.ceil(cy + ry).astype(int)
    w_lo = np.floor(cx - rx).astype(int)
    w_hi = np.ceil(cx + rx).astype(int)
    nrow, ncol = H // BK, W // BK
    buckets = [[[] for _ in range(ncol)] for _ in range(nrow)]
    for n in range(N):
        for bh in range(max(0, h_lo[n] // BK), min(nrow, h_hi[n] // BK + 1)):
            for bw in range(max(0, w_lo[n] // BK), min(ncol, w_hi[n] // BK + 1)):
                buckets[bh][bw].append(n)

    tiles_per_core = []
    for core in range(NCORES):
        blist = []
        for r in range(NBR):
            for c in range(NBC):
                ns = np.asarray(buckets[core * NBR + r][c], dtype=int)
                for s in range(0, len(ns), 128):
                    blist.append((r, c, ns[s:s + 128]))
        blist.sort(key=lambda b: -len(b[2]))
        tiles = []   # [used_partitions, [(r, c, idx, offset), ...]]
        for r, c, idx in blist:
            k = len(idx)
            for t in tiles:
                if t[0] + k <= 128:
                    t[1].append((r, c, idx, t[0]))
                    t[0] += k
                    break
            else:
                tiles.append([k, [(r, c, idx, 0)]])
        tl = [t[1] for t in tiles]
        tl.sort(key=lambda t: -len(t))   # most buckets (=units) first
        tiles_per_core.append(tl)
    return tiles_per_core


def _host_prep(np_inputs):
    """Build per-core device arrays:
      w12  [12, 256 + NT*128] fp16  (cols 0:256 = phi basis, then per-tile
                                     hi/lo quadratic coefficient columns)
      feat [128, NU*128]      fp16  (per-unit dense block feature weights)
    plus the placement map for host-side reassembly."""
    cx, cy, Aq, Bq, Cq, rx, ry, featw = _params(
        np_inputs["xyz_raw"], np_inputs["cholesky_raw"],
        np_inputs["features"], np_inputs["opacity"])
    tiles_per_core = _pack(cx, cy, rx, ry)

    NT = max(len(t) for t in tiles_per_core)
    U = []
    for t in range(NT):
        u = 1
        for core in range(NCORES):
            if t < len(tiles_per_core[core]):
                nb = len(tiles_per_core[core][t])
                u = max(u, -(-(nb * CH) // 128))
        U.append(u)
    NU = sum(U)

    # bucket-centered pixel coords: quarter-integers <= 56.25, exact in fp16
    xs = (np.arange(BK) + 0.5 - BK / 2).astype(np.float64)
    Yg, Xg = np.meshgrid(xs, xs, indexing="ij")
    phi6 = np.stack(
        [Xg * Xg, Yg * Yg, Xg * Yg, Xg, Yg, np.ones_like(Xg)], 0
    ).reshape(6, PXB)
    phi12 = np.concatenate([phi6, phi6], 0).astype(np.float16)

    w12 = np.zeros((NCORES, 12, 256 + NT * 128), dtype=np.float16)
    feat = np.zeros((NCORES, 128, NU * 128), dtype=np.float16)
    w12[:, :, 0:256] = phi12[None]
    placements = []   # per core: list of (tile, unit, pcol, m, ch0, r, c)
    for core in range(NCORES):
        place = []
        uoff = 0
        for t in range(NT):
            tl = tiles_per_core[core][t] if t < len(tiles_per_core[core]) else []
            fw16 = None
            cc = 0
            for r, c, idx, off in tl:
                k = len(idx)
                cxl = cx[idx] - c * BK - BK / 2
                cyl = cy[idx] - (core * NBR + r) * BK - BK / 2
                An, Bn, Cn = Aq[idx], Bq[idx], Cq[idx]
                W6 = np.stack(
                    [
                        An,
                        Cn,
                        Bn,
                        -(2.0 * An * cxl + Bn * cyl),
                        -(2.0 * Cn * cyl + Bn * cxl),
                        An * cxl * cxl + Cn * cyl * cyl + Bn * cxl * cyl,
                    ],
                    0,
                )
                W_hi = W6.astype(np.float16)
                W_lo = (W6 - W_hi.astype(np.float64)).astype(np.float16)
                base = 256 + t * 128 + off
                w12[core, :6, base:base + k] = W_hi
                w12[core, 6:, base:base + k] = W_lo
                fk = featw[idx].astype(np.float16)   # [k, 48]
                ch0 = 0
                while ch0 < CH:
                    unit, pcol = cc // 128, cc % 128
                    m = min(128 - pcol, CH - ch0)
                    feat[core, off:off + k,
                         (uoff + unit) * 128 + pcol:
                         (uoff + unit) * 128 + pcol + m] = fk[:, ch0:ch0 + m]
                    place.append((t, unit, pcol, m, ch0, r, c))
                    ch0 += m
                    cc += m
            uoff += U[t]
        placements.append(place)
    return w12, feat, NT, tuple(U), placements


def _build_program(NT, U, opool_bufs=3):
    import concourse.bacc as bacc
    import concourse.tile as tile
    import concourse.mybir as mybir

    NU = sum(U)
    toff = [0]
    for u in U:
        toff.append(toff[-1] + u * 256)

    nc = bacc.Bacc("TRN2", target_bir_lowering=False, debug=False,
                   num_devices=NCORES)
    w12_ap = nc.dram_tensor("w12", [12, 256 + NT * 128], mybir.dt.float16,
                            kind="ExternalInput").ap()
    feat_ap = nc.dram_tensor("feat", [128, NU * 128], mybir.dt.float16,
                             kind="ExternalInput").ap()
    out_ap = nc.dram_tensor("out", [128, toff[-1]], mybir.dt.float16,
                            kind="ExternalOutput").ap()

    groups = [(g, min(g + 2, NT)) for g in range(0, NT, 2)]
    nuA = sum(U[t] for t in range(groups[0][0], groups[0][1]))

    # greedy ACT/DVE balance for the output copies: ACT starts charged with
    # its exp chain; the final tile is forced to ACT (it is free right after
    # the last exp, while DVE may still have a backlog).
    act_load = NT * 256 * 0.8333 + len(groups) * 185
    dve_load = 727.0
    copy_eng = []
    for t in range(NT):
        cols = U[t] * 256
        ca, cd = cols * 0.8333 + 185, cols * 1.0417 + 125
        if t == NT - 1 or act_load + ca <= dve_load + cd:
            copy_eng.append("act")
            act_load += ca
        else:
            copy_eng.append("dve")
            dve_load += cd

    with tile.TileContext(nc) as tc:
        with ExitStack() as ctx:
            consts = ctx.enter_context(tc.tile_pool(name="consts", bufs=1))
            spool = ctx.enter_context(
                tc.tile_pool(name="sig", bufs=2, space="PSUM"))
            opool = ctx.enter_context(
                tc.tile_pool(name="acc", bufs=opool_bufs, space="PSUM"))
            gpool = ctx.enter_context(tc.tile_pool(name="g", bufs=3))
            stpool = ctx.enter_context(
                tc.tile_pool(name="stage", bufs=NT + 1))

            # PE HAM warmup: dummy matmuls on a zeroed SBUF tile while the
            # input DMAs are in flight, so the p-state ramp completes early.
            dummy = consts.tile([12, 640], mybir.dt.float16)
            nc.vector.memset(dummy, 0)
            for _ in range(2):
                psum_d = spool.tile([128, 512], mybir.dt.float32)
                nc.tensor.matmul(psum_d, dummy[:, 0:128], dummy[:, 128:640],
                                 start=True, stop=True)

            # inputs: w12 split so phi + first-group tiles land first (SP
            # HWDGE); feat chunk A on the ACT HWDGE queue (no waits, so it
            # cannot stall exp dispatch), rest on SP after the w12 chunks.
            w12_sb = consts.tile([12, 256 + NT * 128], mybir.dt.float16)
            CUT = 256 + groups[0][1] * 128
            nc.sync.dma_start(out=w12_sb[:, :CUT], in_=w12_ap[:, :CUT])
            nc.sync.dma_start(out=w12_sb[:, CUT:], in_=w12_ap[:, CUT:])
            phi_sb = w12_sb[:, 0:256]
            feat_sb = consts.tile([128, NU * 128], mybir.dt.float16)
            nc.scalar.dma_start(out=feat_sb[:, :nuA * 128],
                                in_=feat_ap[:, :nuA * 128])
            nc.sync.dma_start(out=feat_sb[:, nuA * 128:],
                              in_=feat_ap[:, nuA * 128:])

            for t0, t1 in groups:
                ntl = t1 - t0
                psum_s = spool.tile([128, ntl * 256], mybir.dt.float32)
                for j in range(ntl):
                    nc.tensor.matmul(
                        psum_s[:, j * 256:(j + 1) * 256],
                        w12_sb[:, 256 + (t0 + j) * 128:256 + (t0 + j + 1) * 128],
                        phi_sb, start=True, stop=True)
                g = gpool.tile([128, ntl * 256], mybir.dt.float16)
                nc.scalar.activation(
                    g, psum_s, mybir.ActivationFunctionType.Exp,
                    bias=0.0, scale=-1.0)
                for j in range(ntl):
                    t = t0 + j
                    psum_o = opool.tile([128, U[t] * 256], mybir.dt.float32)
                    stage = stpool.tile([128, U[t] * 256], mybir.dt.float16)
                    for u in range(U[t]):
                        nc.tensor.matmul(
                            psum_o[:, u * 256:(u + 1) * 256],
                            feat_sb[:, (toff[t] // 256 + u) * 128:
                                    (toff[t] // 256 + u + 1) * 128],
                            g[:, j * 256:(j + 1) * 256],
                            start=True, stop=True)
                        if t == NT - 1:
                            # tail tile: per-unit copy + DMA, short tail
                            nc.scalar.activation(
                                stage[:, u * 256:(u + 1) * 256],
                                psum_o[:, u * 256:(u + 1) * 256],
                                mybir.ActivationFunctionType.Copy)
                            nc.sync.dma_start(
                                out=out_ap[:, toff[t] + u * 256:
                                           toff[t] + (u + 1) * 256],
                                in_=stage[:, u * 256:(u + 1) * 256])
                    if t < NT - 1:
                        if copy_eng[t] == "act":
                            nc.scalar.activation(
                                stage, psum_o,
                                mybir.ActivationFunctionType.Copy)
                        else:
                            nc.vector.tensor_copy(stage, psum_o)
                        nc.sync.dma_start(
                            out=out_ap[:, toff[t]:toff[t + 1]], in_=stage)
    nc.compile()
    return nc


def _get_program(NT, U):
    key = (NT, U)
    if key not in _cached:
        try:
            _cached[key] = _build_program(NT, U, opool_bufs=3)
        except ValueError:
            _cached[key] = _build_program(NT, U, opool_bufs=2)
    return _cached[key]


def kernel(xyz_raw, cholesky_raw, features, opacity):
    from concourse.bass_utils import run_bass_kernel_spmd

    np_inputs = {
        "xyz_raw": np.asarray(xyz_raw, dtype=np.float32),
        "cholesky_raw": np.asarray(cholesky_raw, dtype=np.float32),
        "features": np.asarray(features, dtype=np.float32),
        "opacity": np.asarray(opacity, dtype=np.float32),
    }
    w12, feat, NT, U, placements = _host_prep(np_inputs)
    nc = _get_program(NT, U)

    in_maps = [{"w12": w12[core], "feat": feat[core]}
               for core in range(NCORES)]
    res = run_bass_kernel_spmd(nc, in_maps, core_ids=list(range(NCORES)))

    toff = [0]
    for u in U:
        toff.append(toff[-1] + u * 256)
    out = np.zeros((CH, H, W), dtype=np.float32)
    for core in range(NCORES):
        strips = res.results[core]["out"].astype(np.float32)  # [128, TOT]
        for t, unit, pcol, m, ch0, r, c in placements[core]:
            sl = strips[pcol:pcol + m,
                        toff[t] + unit * 256:toff[t] + unit * 256 + 256]
            out[ch0:ch0 + m,
                (core * NBR + r) * BK:(core * NBR + r + 1) * BK,
                c * BK:(c + 1) * BK] += sl.reshape(m, BK, BK)
    return out.reshape(C, 3, H, W)
